# revision 13
# baseline (speedup 1.0000x reference)
"""MoE layer (top-2 routing, E=8 experts) on 8 Trainium2 NeuronCores.

Strategy (expert-parallel, per sharding hint):
 - Host computes the router (softmax over x@Wg+bg, top-2) and dispatches
   each (token, gate) pair to its expert's core: core e gets the tokens
   routed to expert e (gathered, transposed, zero-padded to a common
   capacity C).
 - Matmuls run as fp8e4 (e4m3) DoubleRow-mode matmuls (K=256 per pass,
   0.5 cycles/row -> 2x the fp32r/bf16 rate).  Full fp32-like accuracy is
   recovered with a hi/lo residual decomposition of both operands and a
   3-term product accumulated in the same fp32 PSUM group:
       a @ b ~= a_hi@b_hi + a_lo@b_hi + a_hi@b_lo      (drops only lo*lo)
   where v_hi = fp8(v), v_lo = fp8(v - v_hi)  (7+ effective mantissa bits).
 - Core e computes for expert e:
       hb = bf16(Gelu((1/S1) * [3-term fp8 x@W1 psum] + b1))  (ACT)
       hh = fp8(hb)                                           (DVE)
       hl = fp8(hb - hh)                                      (DVE)
       y  = [3-term fp8 h@W2 psum] * gates'                   (DVE+Pool evict;
            gates' = gate/SW2 folds away all the quant scales)
 - One continuous software pipeline across token blocks: mm1 for chunk g
   runs LA chunks ahead of the mm2 stream, so a block's mm2 drain overlaps
   the next block's mm1 and the PE never waits at block boundaries.
 - All DMAs are laid out host-side so every transfer is contiguous per
   partition, and streams are split across independent queues: weights +
   consts on SP/ACT HWDGE, x and y on the Pool SWDGE path.
 - Host scatter-adds the per-expert outputs back into [N, D] and adds the
   (separable) b2 term: sum_k gate_k * b2[e_k].
"""

import numpy as np
import ml_dtypes

B, T, D = 4, 2048, 768
E, F, TOPK = 8, 4 * 768, 2
N = B * T
P = 128
NCORES = 8

E4 = ml_dtypes.float8_e4m3
SX = 32.0       # x scale before fp8 (max |x| ~ 5.1 -> 164 < 240)
SW1 = 1024.0    # W1 scale (max ~0.11 -> 111)
SW2 = 1024.0    # W2 scale (h is used unscaled: max ~2.9, fits fp8 range)

_nc_cache = {}


def _blocks_of(C):
    assert C % 128 == 0 and C >= 256
    b384, rem = divmod(C, 384)
    if rem == 0:
        blocks = [384] * b384
    elif rem == 128:
        blocks = [384] * (b384 - 1) + [256, 256]
    else:
        blocks = [384] * b384 + [256]
    assert sum(blocks) == C
    return blocks


def _route(x_flat, Wg, bg):
    """Replicate reference routing: softmax gates, top-2 (ties -> lower idx)."""
    logits = x_flat.astype(np.float64) @ Wg.astype(np.float64) + bg.astype(np.float64)
    logits -= logits.max(axis=-1, keepdims=True)
    eg = np.exp(logits)
    gates = eg / eg.sum(axis=-1, keepdims=True)          # [N, E] f64
    top2 = np.argsort(-gates, axis=-1, kind="stable")[:, :TOPK]   # [N, 2]
    g2 = np.take_along_axis(gates, top2, axis=-1).astype(np.float32)
    return top2, g2


def _hilo_pack(a, s):
    """a: [K, M] with contraction along rows.  Scale by s, split into fp8
    hi/lo, pack each as [128, K//256, 2, M] (partition, double-tile,
    k-tile, col) matching the DoubleRow SBUF layout."""
    sc = a * np.float32(s)
    hi = sc.astype(E4)
    lo = (sc - hi.astype(np.float32)).astype(E4)

    def pack(v):
        nkd = v.shape[0] // 256
        return np.ascontiguousarray(
            v.reshape(nkd, 2, P, v.shape[1]).transpose(2, 0, 1, 3))

    return pack(hi), pack(lo)


def _build_nc(C, LA=3):
    import concourse.bacc as bacc
    import concourse.mybir as mybir
    import concourse.tile as tile

    f32 = mybir.dt.float32
    bf16 = mybir.dt.bfloat16
    fp8 = mybir.dt.float8e4
    Gelu = mybir.ActivationFunctionType.Gelu
    Copy = mybir.ActivationFunctionType.Copy
    DR = mybir.MatmulPerfMode.DoubleRow

    KO2 = F // P          # 24 h chunks per block
    ND1 = D // 256        # 3 double-k-tiles for x@W1
    ND2 = F // 256        # 12 double-k-tiles for h@W2
    NQ = 4                # weight DMA staggered in 4 quarters
    FQ = F // NQ          # 768
    DH = 2                # output D split (psum tile free dim 384)
    DHW = D // DH
    inv_S1 = 1.0 / (SX * SW1)
    blocks = _blocks_of(C)
    NBLK = len(blocks)
    tok0s = [sum(blocks[:b]) for b in range(NBLK)]

    nc = bacc.Bacc("TRN2", target_bir_lowering=False)

    # x is packed per block so every per-block DMA is one contiguous
    # per-partition segment of 6*TBl bytes.
    xh = nc.dram_tensor("xh", [P, ND1 * 2 * C], fp8, kind="ExternalInput")
    xl = nc.dram_tensor("xl", [P, ND1 * 2 * C], fp8, kind="ExternalInput")
    w1h = nc.dram_tensor("w1h", [P, ND1, 2, F], fp8, kind="ExternalInput")
    w1l = nc.dram_tensor("w1l", [P, ND1, 2, F], fp8, kind="ExternalInput")
    w2h = nc.dram_tensor("w2h", [P, ND2, 2, D], fp8, kind="ExternalInput")
    w2l = nc.dram_tensor("w2l", [P, ND2, 2, D], fp8, kind="ExternalInput")
    b1 = nc.dram_tensor("b1", [P, KO2], f32, kind="ExternalInput")
    gates = nc.dram_tensor("gates", [P, C // P], f32, kind="ExternalInput")
    y = nc.dram_tensor("y", [C, D], bf16, kind="ExternalOutput")

    with tile.TileContext(nc) as tc:
        with (
            tc.tile_pool(name="wpool", bufs=1) as wpool,
            tc.tile_pool(name="xpool", bufs=2) as xpool,
            tc.tile_pool(name="hpool", bufs=2) as hpool,
            tc.tile_pool(name="ypool", bufs=2) as ypool,
            tc.tile_pool(name="psum1", bufs=2, space="PSUM") as psum1,
            tc.tile_pool(name="psumy", bufs=1, space="PSUM") as psumy,
        ):
            # Tiny constants on the ACT HWDGE queue (done in ~2us, before
            # the first ACT/evict needs them; doesn't delay weights on SP).
            b1_sb = wpool.tile([P, KO2], f32, tag="b1", name="b1_sb")
            nc.scalar.dma_start(b1_sb[:], b1[:, :])
            gates_sb = wpool.tile([P, C // P], f32, tag="gates",
                                  name="gates_sb")
            nc.scalar.dma_start(gates_sb[:], gates[:, :])

            # Weights (SP HWDGE queue), in first-block consumption order.
            # Quarter 0 is split per double-k-tile so the PE's first matmul
            # waits only on a small transfer.
            w1h_t, w1l_t, w2h_t, w2l_t = [], [], [], []

            def w_dma(pfx, src, q, nkd, fw, split):
                tiles = []
                ks = range(nkd) if split else [None]
                for k in ks:
                    t = wpool.tile([P, 1 if split else nkd, 2, fw], fp8,
                                   tag=f"{pfx}{q}_{k}", name=f"{pfx}{q}_{k}")
                    if pfx.startswith("w1"):
                        src_ap = (src[:, k:k + 1, :, q * fw:(q + 1) * fw]
                                  if split else
                                  src[:, :, :, q * fw:(q + 1) * fw])
                    else:
                        k0 = q * nkd
                        src_ap = (src[:, k0 + k:k0 + k + 1, :, :] if split
                                  else src[:, k0:k0 + nkd, :, :])
                    nc.sync.dma_start(t[:], src_ap)
                    tiles.append(t)
                return tiles

            for q in range(NQ):
                split = (q == 0)
                w1h_t.append(w_dma("w1h", w1h, q, ND1, FQ, split))
                w1l_t.append(w_dma("w1l", w1l, q, ND1, FQ, split))
                w2h_t.append(w_dma("w2h", w2h, q, ND2 // NQ, D, split))
                w2l_t.append(w_dma("w2l", w2l, q, ND2 // NQ, D, split))

            def w_ap(tiles, k, s0, s1):
                """Slice [P, 2, s1-s0] of logical k-tile k, col slice."""
                if len(tiles) > 1:
                    return tiles[k][:, 0, :, s0:s1]
                return tiles[0][:, k, :, s0:s1]

            # Per-block state created lazily inside the single global
            # pipeline loop.
            xs = [None] * NBLK
            hs = [None] * NBLK
            ys = [None] * NBLK
            ypsum = [
                [psumy.tile([P, DHW], f32, tag=f"y_{ts}_{dh}",
                            name=f"ypsum_{ts}_{dh}") for dh in range(DH)]
                for ts in range(max(blocks) // P)
            ]

            def load_x(b):
                TBl = blocks[b]
                seg0 = ND1 * 2 * tok0s[b]
                segn = ND1 * 2 * TBl
                xh_sb = xpool.tile([P, ND1, 2, TBl], fp8, tag="xh",
                                   name="xh_sb")
                nc.gpsimd.dma_start(
                    xh_sb[:],
                    xh[:, seg0:seg0 + segn].rearrange(
                        "p (kd t c) -> p kd t c", kd=ND1, t=2))
                xl_sb = xpool.tile([P, ND1, 2, TBl], fp8, tag="xl",
                                   name="xl_sb")
                nc.gpsimd.dma_start(
                    xl_sb[:],
                    xl[:, seg0:seg0 + segn].rearrange(
                        "p (kd t c) -> p kd t c", kd=ND1, t=2))
                xs[b] = (xh_sb, xl_sb)

            load_x(0)
            G = NBLK * KO2
            for g in range(G + LA):
                if g < G:
                    b, i = divmod(g, KO2)
                    TBl = blocks[b]
                    if i == 0 and b + 1 < NBLK and NBLK == 1:
                        pass
                    if i == 0:
                        hs[b] = (
                            [hpool.tile([P, 2, TBl], bf16, tag=f"hb_{k}",
                                        name=f"hb{k}") for k in range(ND2)],
                            [hpool.tile([P, 2, TBl], fp8, tag=f"hh_{k}",
                                        name=f"hh{k}") for k in range(ND2)],
                            [hpool.tile([P, 2, TBl], fp8, tag=f"hl_{k}",
                                        name=f"hl{k}") for k in range(ND2)],
                        )
                    if i == KO2 // 2 and b + 1 < NBLK:
                        load_x(b + 1)   # prefetch next block's tokens
                    xh_sb, xl_sb = xs[b]
                    q, fq = i // (KO2 // NQ), i % (KO2 // NQ)
                    p1 = psum1.tile([P, TBl], f32, tag="p1", name="p1_sb")
                    terms = ((xh_sb, w1h_t[q]), (xl_sb, w1h_t[q]),
                             (xh_sb, w1l_t[q]))
                    for t, (xa, wa) in enumerate(terms):
                        for kd in range(ND1):
                            nc.tensor.matmul(
                                p1[:],
                                lhsT=w_ap(wa, kd, fq * P, (fq + 1) * P),
                                rhs=xa[:, kd, :, :],
                                start=(t == 0 and kd == 0),
                                stop=(t == 2 and kd == ND1 - 1),
                                perf_mode=DR,
                            )
                    hb_sb, hh_sb, hl_sb = hs[b]
                    kp, tp = i // 2, i % 2
                    nc.scalar.activation(hb_sb[kp][:, tp, :], p1[:], Gelu,
                                         bias=b1_sb[:, i:i + 1],
                                         scale=inv_S1)
                    nc.vector.tensor_copy(hh_sb[kp][:, tp, :],
                                          hb_sb[kp][:, tp, :])
                    nc.vector.tensor_sub(hl_sb[kp][:, tp, :],
                                         hb_sb[kp][:, tp, :],
                                         hh_sb[kp][:, tp, :])
                j = g - LA
                if j >= 0 and j % 2 == 1:
                    jb, ji = divmod(j, KO2)
                    TBl = blocks[jb]
                    TS = TBl // P
                    kd = ji // 2
                    q = kd // (ND2 // NQ)
                    kq = kd % (ND2 // NQ)
                    if kd == 0:
                        ys[jb] = ypool.tile([P, TS, D], bf16, tag="y",
                                            name="y_sb")
                    _, hh_sb, hl_sb = hs[jb]
                    terms2 = ((hh_sb[kd], w2h_t[q]), (hh_sb[kd], w2l_t[q]),
                              (hl_sb[kd], w2h_t[q]))
                    for ts in range(TS):
                        for t, (ha, wa) in enumerate(terms2):
                            for dh in range(DH):
                                nc.tensor.matmul(
                                    ypsum[ts][dh][:],
                                    lhsT=ha[:, :, ts * P:(ts + 1) * P],
                                    rhs=w_ap(wa, kq, dh * DHW, (dh + 1) * DHW),
                                    start=(kd == 0 and t == 0),
                                    stop=(kd == ND2 - 1 and t == 2),
                                    perf_mode=DR,
                                )
                    if kd == ND2 - 1:
                        y_sb = ys[jb]
                        tok0 = tok0s[jb]
                        mo0 = tok0 // P
                        for ts in range(TS):
                            g_ap = gates_sb[:, mo0 + ts: mo0 + ts + 1]
                            # dh halves on different engines; y DMA per half
                            # so the tail drains in parallel
                            nc.vector.tensor_scalar_mul(
                                y_sb[:, ts, 0:DHW], ypsum[ts][0][:], g_ap)
                            nc.sync.dma_start(
                                y[tok0 + ts * P:tok0 + (ts + 1) * P, 0:DHW],
                                y_sb[:, ts, 0:DHW])
                            nc.scalar.activation(
                                y_sb[:, ts, DHW:D], ypsum[ts][1][:],
                                Copy, scale=g_ap)
                            nc.scalar.dma_start(
                                y[tok0 + ts * P:tok0 + (ts + 1) * P, DHW:D],
                                y_sb[:, ts, DHW:D])
    nc.compile()
    return nc


def kernel(x, Wg, bg, W1, b1, W2, b2):
    from concourse.bass_utils import run_bass_kernel_spmd

    x = np.asarray(x, dtype=np.float32)
    Wg = np.asarray(Wg, dtype=np.float32)
    bg = np.asarray(bg, dtype=np.float32)
    W1 = np.asarray(W1, dtype=np.float32)
    b1 = np.asarray(b1, dtype=np.float32)
    W2 = np.asarray(W2, dtype=np.float32)
    b2 = np.asarray(b2, dtype=np.float32)

    x_flat = x.reshape(-1, D)
    top2, g2 = _route(x_flat, Wg, bg)

    # Dispatch: token lists per expert
    idx_e = []
    gate_e = []
    for e in range(E):
        sel = np.nonzero(top2 == e)
        idx_e.append(sel[0].astype(np.int64))                  # token ids
        gate_e.append(g2[sel[0], sel[1]].astype(np.float32))   # their gates
    counts = [len(i) for i in idx_e]
    C = max(max(counts), 256)
    C = ((C + P - 1) // P) * P

    if C not in _nc_cache:
        _nc_cache[C] = _build_nc(C)
    nc = _nc_cache[C]

    blocks = _blocks_of(C)
    ND1 = D // 256

    def x_block_pack(xp):
        # [P, ND1, 2, C] -> per-block contiguous [P, ND1*2*C]
        segs = []
        t0 = 0
        for TBl in blocks:
            segs.append(xp[:, :, :, t0:t0 + TBl].reshape(P, -1))
            t0 += TBl
        return np.ascontiguousarray(np.concatenate(segs, axis=1))

    b1_packed_base = b1.reshape(E, F // P, P).transpose(0, 2, 1)  # [E, P, 24]

    in_maps = []
    for e in range(E):
        n_e = counts[e]
        xTe = np.zeros((D, C), dtype=np.float32)
        if n_e:
            xTe[:, :n_e] = x_flat[idx_e[e]].T
        xh_p, xl_p = _hilo_pack(xTe, SX)
        w1h_p, w1l_p = _hilo_pack(W1[e], SW1)
        w2h_p, w2l_p = _hilo_pack(W2[e], SW2)
        ge = np.zeros((C,), dtype=np.float32)
        ge[:n_e] = gate_e[e] / np.float32(SW2)
        in_maps.append({
            "xh": x_block_pack(xh_p), "xl": x_block_pack(xl_p),
            "w1h": w1h_p, "w1l": w1l_p,
            "w2h": w2h_p, "w2l": w2l_p,
            "b1": np.ascontiguousarray(b1_packed_base[e]),
            "gates": np.ascontiguousarray(
                ge.reshape(C // P, P).T),           # [P, C//P]
        })

    res = run_bass_kernel_spmd(nc, in_maps, core_ids=list(range(NCORES)))

    out = np.zeros((N, D), dtype=np.float32)
    for e in range(E):
        n_e = counts[e]
        if n_e:
            out[idx_e[e]] += res.results[e]["y"][:n_e].astype(np.float32)
    # separable b2 term: sum_k gate_k * b2[e_k]
    if np.any(b2):
        out += g2[:, 0:1] * b2[top2[:, 0]] + g2[:, 1:2] * b2[top2[:, 1]]
    return out.reshape(B, T, D)
